# revision 10
# baseline (speedup 1.0000x reference)
"""Trainium2 Bass kernel for a 2-layer GraphConv GNN + mean-pool + linear.

Reference computation (all fp32):
    h1 = leaky_relu(segsum(w*x[src] -> dst) @ W1_rel + x @ W1_root + b1)
    h2 = leaky_relu(segsum(w*h1[src] -> dst) @ W2_rel + h1 @ W2_root + b2)
    pooled = segment_mean(h2, batch, 512)
    out = pooled @ Wl_root + bl            # [512, 8]

The per-edge gather x[src] / h1[src] is the bottleneck: any data-dependent
DMA costs ~5-7ns/edge of software descriptor generation on a GPSIMD Q7 core
pair. This version uses the dma_gather custom-ucode instruction with
~1024-index calls rotated across 4 SWDGE queues, which spreads descriptor
generation over all 4 Q7 core pairs (the ucode routes each call to pair
`queue_num`), overlapping generation ~3x vs a single queue.

dma_gather constraints and how they're met:
  - elem_size_bytes % 256 == 0  -> fp32 rows of 64 features (256B).
  - int16 indices (< 32768)     -> gather through 4 strided table views
    (elem_step=256 elems = 4 rows, base offset r rows); idx = src//4 with
    edges grouped per dst-block by residue r = src%4. Works for both tables
    since NPC=12500 and NPAD=12544 are divisible by 4 (so src%4 residues
    are preserved in the padded h1 layout).
  - ~1024 idx max per call (Q7 scratch) -> one call per (block-pair,
    residue), covering both blocks' chunks.
  - indices wrapped [i%16, i//16] into 16 partitions, replicated 8x down.

Distribution (8 NeuronCores): nodes in contiguous ranges of 12500 per core;
edges on the dst-owning core; scatter-add to dst slots via one-hot matmuls
(DVE builds onehot[e,s] = (s == dst_in_block[e]) * w[e], TensorE contracts
with the gathered rows into a feature-major PSUM tile); h1 exchanged with an
AllGather; per-graph pooling via one-hot matmul; trivial overlap-add + mean
+ final 64x8 linear on host.
"""

import math

import numpy as np

import concourse.bacc as bacc
import concourse.bass as bass
import concourse.mybir as mybir
import concourse.tile as tile
from concourse.bass_utils import run_bass_kernel_spmd

F32 = mybir.dt.float32
BF16 = mybir.dt.bfloat16
I16 = mybir.dt.int16
ALU = mybir.AluOpType
ACTF = mybir.ActivationFunctionType

NRES = 4      # residue groups (table views); int16 idx limit, elem_step=4
MAX_CALL = 8  # max chunks per dma_gather call (1024 idxs; Q7 scratch limit)
NQ = 4        # SWDGE queues (Q7 core pairs) to rotate gather calls over


class Cfg:
    def __init__(self, n_nodes, n_edges, d, n_graphs, n_cores=8, block=128):
        assert n_nodes % n_cores == 0
        self.N = n_nodes
        self.E = n_edges
        self.D = d
        self.G = n_graphs
        self.CORES = n_cores
        self.BLOCK = block
        self.NPC = n_nodes // n_cores                    # nodes per core
        self.NB = math.ceil(self.NPC / block)            # blocks per core
        self.NPAD = self.NB * block                      # padded nodes/core
        self.NBP = self.NB // 2                          # block pairs
        self.LEAKY = 0.01
        assert self.NB % 2 == 0
        assert self.NPC % NRES == 0 and self.NPAD % NRES == 0


REAL_CFG = Cfg(100000, 1250000, 64, 512)


def _dma_gather_any(nc, out_ap, in_ap, idxs_ap, num_idxs, elem_size,
                    elem_step, queue_num):
    """dma_gather with the 256B-element restriction relaxed to 128B.

    Mirrors concourse.bass.BassGpSimd.dma_gather (non-transpose, HBM source);
    the ucode (dma_gather.cpp) computes descriptor lengths from
    elem_size*dtype_size generically -- only stride_bytes must divide by 256.
    """
    import concourse.ap_utils as ap_utils
    gp = nc.gpsimd
    gp._assert_queue_num(queue_num)
    assert idxs_ap.dtype == mybir.dt.int16
    assert in_ap.dtype == out_ap.dtype
    elem_size_bytes = elem_size * mybir.dt.size(in_ap.dtype)
    assert elem_size_bytes % 128 == 0
    assert ap_utils.ap_is_contiguous(in_ap.ap[1:])
    assert ap_utils.ap_is_contiguous(out_ap.ap[1:])
    assert ap_utils.ap_is_contiguous(idxs_ap.ap[1:])
    assert in_ap.ap[-1][1] == out_ap.ap[-1][1] == elem_size
    assert out_ap.ap[0][1] * out_ap.ap[1][1] == ((num_idxs + 127) // 128) * 128
    assert in_ap.ap[0][0] == elem_step
    stride_bytes = elem_step * mybir.dt.size(in_ap.dtype)
    assert stride_bytes % 256 == 0
    _in_ap = gp.lower_ap_dma(in_ap, for_custom_bir_dma=True)
    _idxs_ap = gp.lower_ap(idxs_ap)
    _out_ap = gp.lower_ap(out_ap)
    return gp.add_instruction(
        mybir.InstDMAGatherAnt(
            name=nc.get_next_instruction_name(),
            ins=[*_in_ap, _idxs_ap,
                 gp.lower_val_access(gp.to_reg(num_idxs))],
            outs=[_out_ap],
            transpose=False,
            num_idxs=num_idxs,
            elem_size=elem_size,
            stride_bytes_256=stride_bytes // 256,
            gen_mode=0,
            single_packet=True,
            queue_num=queue_num,
            sbuf_tokens_per_rank=0,
            sbuf_free_dim_per_rank=0,
            sbuf_free_dim_pad_per_rank=0,
            sbuf_byte_offset=0,
        ))


# ---------------------------------------------------------------------------
# Host-side preprocessing: shard edges by dst core, group per (dst-block-pair,
# src%4, block-half), pad each group to chunks of 128, build int16 index
# streams wrapped for dma_gather.
# ---------------------------------------------------------------------------

def _wrap16(idx):
    """[n] int -> [128, n//16] int16: i at [i%16, i//16], replicated 8x."""
    n = len(idx)
    w = np.ascontiguousarray(idx.reshape(n // 16, 16).T).astype(np.int16)
    return np.tile(w, (8, 1))


def preprocess(cfg, x, edge_index, weights, batch):
    N, E, D, CORES = cfg.N, cfg.E, cfg.D, cfg.CORES
    NPC, NB, NPAD, BLOCK = cfg.NPC, cfg.NB, cfg.NPAD, cfg.BLOCK
    NBP = cfg.NBP

    src = np.asarray(edge_index[0], dtype=np.int64)
    dst = np.asarray(edge_index[1], dtype=np.int64)
    w = np.asarray(weights, dtype=np.float32)
    batch = np.asarray(batch, dtype=np.int64)

    core_of = dst // NPC
    ld = dst - core_of * NPC
    blk = ld // BLOCK
    dib = ld - blk * BLOCK
    res = src % NRES
    bp = blk // 2
    half = blk % 2
    order = np.lexsort((half, res, bp, core_of))
    src_s, w_s = src[order], w[order]
    core_s, bp_s, res_s, half_s, dib_s = (
        core_of[order], bp[order], res[order], half[order],
        dib[order].astype(np.float32))

    # group = (bp, r, half) in that order
    gl = (bp_s * NRES + res_s) * 2 + half_s
    gid = core_s * (NBP * NRES * 2) + gl
    NG = CORES * NBP * NRES * 2
    counts = np.bincount(gid, minlength=NG).reshape(CORES, NBP * NRES * 2)
    # chunks per group: max over cores (SPMD identical program)
    K = np.maximum(1, -(-counts.max(axis=0) // BLOCK))  # [NBP*NRES*2]
    cs = np.concatenate([[0], np.cumsum(K)])            # chunk col offsets
    C = int(cs[-1])                                     # chunks per core

    first = np.concatenate([[0], np.cumsum(counts.reshape(-1))])[:-1]
    rank = np.arange(E, dtype=np.int64) - first[gid]
    slot = cs[gl] * BLOCK + rank                        # slot within core

    q1 = np.zeros((CORES, C * BLOCK), dtype=np.int16)
    q2 = np.zeros((CORES, C * BLOCK), dtype=np.int16)
    dibp = np.full((CORES, C * BLOCK), -1.0, dtype=np.float32)
    wp = np.zeros((CORES, C * BLOCK), dtype=np.float32)
    q1[core_s, slot] = (src_s // NRES).astype(np.int16)
    sp = (src_s // NPC) * NPAD + (src_s % NPC)          # padded-layout id
    q2[core_s, slot] = (sp // NRES).astype(np.int16)
    dibp[core_s, slot] = dib_s
    wp[core_s, slot] = w_s

    def tocol(a):                       # [C*128] -> [128, C] (col per chunk)
        return np.ascontiguousarray(a.reshape(C, BLOCK).T)

    import jax.numpy as jnp
    x_bf16 = np.asarray(jnp.asarray(x, dtype=jnp.bfloat16))

    g_base = batch[np.arange(CORES) * NPC]
    in_maps = []
    for c in range(CORES):
        xs = x[c * NPC:(c + 1) * NPC]
        xT = np.zeros((D, NPAD), dtype=np.float32)
        xT[:, :NPC] = xs.T
        gs = np.full(NPAD, -1.0, dtype=np.float32)
        gs[:NPC] = (batch[c * NPC:(c + 1) * NPC] - g_base[c]).astype(
            np.float32)
        assert gs.max() < 128.0, "graph span per core exceeds 128"
        in_maps.append({
            "x": np.ascontiguousarray(x_bf16),
            "xT": xT,
            "idx1": _wrap16(q1[c]),
            "idx2": _wrap16(q2[c]),
            "dib": tocol(dibp[c]),
            "wgt": tocol(wp[c]),
            "gslot": np.ascontiguousarray(gs.reshape(NB, BLOCK).T),
        })
    return in_maps, K.reshape(NBP, NRES, 2).tolist(), cs.tolist(), g_base


# ---------------------------------------------------------------------------
# Bass program
# ---------------------------------------------------------------------------

def build_nc(cfg, K, cs, reps=1):
    """K: [NBP][NRES][2] chunks per (block-pair, residue, half); cs: chunk
    col offsets. reps>1 repeats the computation (timing harness: the delta
    between a 2x and 1x program cancels dispatch overhead)."""
    N, D, CORES = cfg.N, cfg.D, cfg.CORES
    NB, NPAD, NBP = cfg.NB, cfg.NPAD, cfg.NBP
    C = cs[-1]

    nc = bacc.Bacc("TRN2", target_bir_lowering=False, debug=False,
                   num_devices=CORES, num_swdge_queues=NQ)

    x_d = nc.dram_tensor("x", [N, D], BF16, kind="ExternalInput")
    xT_d = nc.dram_tensor("xT", [D, NPAD], F32, kind="ExternalInput")
    idx1_d = nc.dram_tensor("idx1", [128, C * 8], I16, kind="ExternalInput")
    idx2_d = nc.dram_tensor("idx2", [128, C * 8], I16, kind="ExternalInput")
    dib_d = nc.dram_tensor("dib", [128, C], F32, kind="ExternalInput")
    wgt_d = nc.dram_tensor("wgt", [128, C], F32, kind="ExternalInput")
    gslot_d = nc.dram_tensor("gslot", [128, NB], F32, kind="ExternalInput")
    w1c_d = nc.dram_tensor("W1c", [2 * D, D], F32, kind="ExternalInput")
    w2c_d = nc.dram_tensor("W2c", [2 * D, D], F32, kind="ExternalInput")
    b1_d = nc.dram_tensor("b1", [D, 1], F32, kind="ExternalInput")
    b2_d = nc.dram_tensor("b2", [D, 1], F32, kind="ExternalInput")
    iota_d = nc.dram_tensor("iota", [128, 128], F32, kind="ExternalInput")
    id64_d = nc.dram_tensor("id64", [D, D], F32, kind="ExternalInput")

    pool_d = nc.dram_tensor("pool", [128, D], F32, kind="ExternalOutput")

    h1_local = nc.dram_tensor("h1_local", [NPAD, D], BF16)
    h1_full = nc.dram_tensor("h1_full", [NPAD * CORES, D], BF16,
                             addr_space="Shared")

    with tile.TileContext(nc) as tc:
        with (
            tc.tile_pool(name="persist", bufs=1) as pp,
            tc.tile_pool(name="work", bufs=6) as wp,
            tc.tile_pool(name="gat", bufs=6) as gp,
            tc.tile_pool(name="agg", bufs=3, space="PSUM") as aggp,
            tc.tile_pool(name="ps", bufs=2, space="PSUM") as psp,
            tc.tile_pool(name="pool1", bufs=1, space="PSUM") as pool1,
        ):
            xT_s = pp.tile([D, NPAD], F32, tag="xT")
            h1T_s = pp.tile([D, NPAD], F32, tag="h1T")
            idx1_s = pp.tile([128, C * 8], I16, tag="idx1")
            idx2_s = pp.tile([128, C * 8], I16, tag="idx2")
            dib_s = pp.tile([128, C], F32, tag="dib")
            wgt_s = pp.tile([128, C], F32, tag="wgt")
            gslot_s = pp.tile([128, NB], F32, tag="gslot")
            w1c_s = pp.tile([2 * D, D], F32, tag="w1c")
            w2c_s = pp.tile([2 * D, D], F32, tag="w2c")
            b1_s = pp.tile([D, 1], F32, tag="b1")
            b2_s = pp.tile([D, 1], F32, tag="b2")
            iota_s = pp.tile([128, 128], F32, tag="iota")
            id64_s = pp.tile([D, D], F32, tag="id64")

            for t, d in [(xT_s, xT_d), (idx1_s, idx1_d), (idx2_s, idx2_d),
                         (dib_s, dib_d), (wgt_s, wgt_d), (gslot_s, gslot_d),
                         (w1c_s, w1c_d), (w2c_s, w2c_d), (b1_s, b1_d),
                         (b2_s, b2_d), (iota_s, iota_d), (id64_s, id64_d)]:
                nc.sync.dma_start(out=t[:], in_=d[:, :])

            pool_ps = pool1.tile([128, D], F32, tag="pool")
            qctr = [0]

            LAG = 2  # blocks of tail pipelining (PSUM agg bufs = LAG+1)

            def layer(idx_s, table_d, wc_s, b_s, xfm_s, last_stage):
                # strided views: row stride 4 rows (1024B), base offset r rows
                tabv = table_d[:, :].rearrange("(a b) f -> a (b f)", b=NRES)
                pending = []

                def tail(b, agg_ps):
                    # emitted LAG blocks late so the DVE tail of block b
                    # hides under the PE agg matmuls of blocks b+1..b+LAG
                    cat = wp.tile([2 * D, 128], F32, tag="cat")
                    nc.vector.tensor_copy(out=cat[0:D, :], in_=agg_ps[:])
                    nc.vector.tensor_copy(
                        out=cat[D:2 * D, :],
                        in_=xfm_s[:, b * 128:(b + 1) * 128])
                    z_ps = psp.tile([D, 128], F32, tag="z")
                    nc.tensor.matmul(out=z_ps[:], lhsT=wc_s[:],
                                     rhs=cat[:], start=True, stop=True)
                    last_stage(b, z_ps, b_s)

                for bp in range(NBP):
                    # one gather call per residue covering both half-blocks
                    # (chunk cols for (bp, r, 0) and (bp, r, 1) are adjacent)
                    tiles = {}
                    for r in range(NRES):
                        k1, k2 = K[bp][r][0], K[bp][r][1]
                        c0 = cs[(bp * NRES + r) * 2]
                        segs = ([(c0, k1 + k2)] if k1 + k2 <= MAX_CALL
                                else [(c0, k1), (c0 + k1, k2)])
                        gts = []
                        for (cc0, kk) in segs:
                            g = gp.tile([128, kk * 64], BF16, tag="g")
                            gv = g[:].rearrange("p (c f) -> p c f", c=kk)
                            _dma_gather_any(
                                nc, gv, tabv[:, r * 64:(r + 1) * 64],
                                idx_s[:, cc0 * 8:(cc0 + kk) * 8],
                                kk * 128, 64, NRES * 64, qctr[0] % NQ)
                            qctr[0] += 1
                            gts.append((g, cc0, kk))
                        tiles[r] = gts
                    for half in range(2):
                        b = 2 * bp + half
                        agg_ps = aggp.tile([D, 128], F32, tag="agg")
                        total = sum(K[bp][r][half] for r in range(NRES))
                        done = 0
                        for r in range(NRES):
                            cstart = cs[(bp * NRES + r) * 2] \
                                + (K[bp][r][0] if half else 0)
                            for j in range(K[bp][r][half]):
                                cc = cstart + j
                                # locate gather tile holding chunk cc
                                for (g, gc0, gkk) in tiles[r]:
                                    if gc0 <= cc < gc0 + gkk:
                                        col = cc - gc0
                                        break
                                oh = wp.tile([128, 128], BF16, tag="oh")
                                nc.vector.tensor_scalar(
                                    out=oh[:], in0=iota_s[:],
                                    scalar1=dib_s[:, cc:cc + 1],
                                    scalar2=wgt_s[:, cc:cc + 1],
                                    op0=ALU.is_equal, op1=ALU.mult)
                                nc.tensor.matmul(
                                    out=agg_ps[:],
                                    lhsT=g[:, col * 64:(col + 1) * 64],
                                    rhs=oh[:],
                                    start=(done == 0),
                                    stop=(done == total - 1))
                                done += 1
                        pending.append((b, agg_ps))
                        if len(pending) > LAG:
                            tail(*pending.pop(0))
                while pending:
                    tail(*pending.pop(0))

            def leaky(dst_ap, z_ps, b_s):
                zb = wp.tile([D, 128], F32, tag="zb")
                nc.vector.tensor_scalar_add(out=zb[:], in0=z_ps[:],
                                            scalar1=b_s[:, 0:1])
                t = wp.tile([D, 128], F32, tag="zt")
                nc.vector.tensor_scalar_mul(out=t[:], in0=zb[:],
                                            scalar1=cfg.LEAKY)
                nc.vector.tensor_tensor(out=dst_ap, in0=zb[:], in1=t[:],
                                        op=ALU.max)

            def l1_tail(b, z_ps, b_s):
                hslice = h1T_s[:, b * 128:(b + 1) * 128]
                leaky(hslice, z_ps, b1_s)
                t_ps = psp.tile([128, D], F32, tag="tp")
                nc.tensor.transpose(out=t_ps[:], in_=hslice,
                                    identity=id64_s[:])
                h1nm = wp.tile([128, D], BF16, tag="h1nm")
                nc.vector.tensor_copy(out=h1nm[:], in_=t_ps[:])
                nc.sync.dma_start(out=h1_local[b * 128:(b + 1) * 128, :],
                                  in_=h1nm[:])

            for _rep in range(reps):
                layer(idx1_s, x_d, w1c_s, b1_s, xT_s, l1_tail)

                nc.gpsimd.collective_compute(
                    "AllGather",
                    ALU.bypass,
                    replica_groups=[list(range(CORES))],
                    ins=[h1_local.ap()],
                    outs=[h1_full.ap()],
                )

                def l2_tail(b, z_ps, b_s):
                    h2fm = wp.tile([D, 128], F32, tag="h2fm")
                    leaky(h2fm[:], z_ps, b2_s)
                    t_ps = psp.tile([128, D], F32, tag="tp")
                    nc.tensor.transpose(out=t_ps[:], in_=h2fm[:],
                                        identity=id64_s[:])
                    h2nm = wp.tile([128, D], F32, tag="h2nm")
                    nc.vector.tensor_copy(out=h2nm[:], in_=t_ps[:])
                    ph = wp.tile([128, 128], F32, tag="ph")
                    nc.vector.tensor_scalar(
                        out=ph[:], in0=iota_s[:],
                        scalar1=gslot_s[:, b:b + 1], scalar2=None,
                        op0=ALU.is_equal)
                    nc.tensor.matmul(out=pool_ps[:], lhsT=ph[:],
                                     rhs=h2nm[:],
                                     start=(b == 0), stop=(b == NB - 1))

                layer(idx2_s, h1_full, w2c_s, b2_s, h1T_s, l2_tail)

            pool_s = wp.tile([128, D], F32, tag="pools")
            nc.scalar.activation(out=pool_s[:], in_=pool_ps[:],
                                 func=ACTF.Copy)
            nc.sync.dma_start(out=pool_d[:, :], in_=pool_s[:])

    nc.compile()
    return nc


# ---------------------------------------------------------------------------
# Entry point
# ---------------------------------------------------------------------------

_CACHE = {}


def _common_inputs(cfg, W1_root, W1_rel, W2_root, W2_rel, b1, b2):
    D = cfg.D
    return {
        "W1c": np.concatenate([W1_rel, W1_root], axis=0).astype(np.float32),
        "W2c": np.concatenate([W2_rel, W2_root], axis=0).astype(np.float32),
        "b1": np.ascontiguousarray(b1.reshape(D, 1).astype(np.float32)),
        "b2": np.ascontiguousarray(b2.reshape(D, 1).astype(np.float32)),
        "iota": np.broadcast_to(np.arange(128, dtype=np.float32),
                                (128, 128)).copy(),
        "id64": np.eye(D, dtype=np.float32),
    }


def _kkey(K):
    return tuple(tuple(tuple(h) for h in r) for r in K)


def run(cfg, inputs, trace=False):
    x = np.asarray(inputs["x_embeddings"], dtype=np.float32)
    in_maps, K, cs, g_base = preprocess(
        cfg, x, inputs["edge_index"], inputs["weights"], inputs["batch"])
    common = _common_inputs(cfg, inputs["W1_root"], inputs["W1_rel"],
                            inputs["W2_root"], inputs["W2_rel"],
                            inputs["b1"], inputs["b2"])
    for m in in_maps:
        m.update(common)

    key = (cfg.N, cfg.E, _kkey(K))
    if key not in _CACHE:
        _CACHE[key] = build_nc(cfg, K, cs)
    nc = _CACHE[key]

    res = run_bass_kernel_spmd(nc, in_maps, core_ids=list(range(cfg.CORES)),
                               trace=trace)

    batch = np.asarray(inputs["batch"], dtype=np.int64)
    counts = np.bincount(batch, minlength=cfg.G).astype(np.float32)
    pooled = np.zeros((cfg.G + 128, cfg.D), dtype=np.float32)
    for c in range(cfg.CORES):
        pooled[g_base[c]:g_base[c] + 128] += res.results[c]["pool"]
    pooled = pooled[:cfg.G] / np.maximum(counts, 1.0)[:, None]
    out = pooled @ np.asarray(inputs["Wl_root"], dtype=np.float32)
    out = out + np.asarray(inputs["bl"], dtype=np.float32)
    return out.astype(np.float32), res


def kernel(**inputs) -> np.ndarray:
    out, _ = run(REAL_CFG, inputs, trace=False)
    return out


# revision 11
# speedup vs baseline: 1.7267x; 1.7267x over previous
"""Trainium2 Bass kernel for a 2-layer GraphConv GNN + mean-pool + linear.

Reference computation (all fp32):
    h1 = leaky_relu(segsum(w*x[src] -> dst) @ W1_rel + x @ W1_root + b1)
    h2 = leaky_relu(segsum(w*h1[src] -> dst) @ W2_rel + h1 @ W2_root + b2)
    pooled = segment_mean(h2, batch, 512)
    out = pooled @ Wl_root + bl            # [512, 8]

The per-edge gather x[src] / h1[src] is the bottleneck: any data-dependent
DMA costs ~5-7ns/edge of software descriptor generation on a GPSIMD Q7 core
pair. This version uses the dma_gather custom-ucode instruction with
~1024-index calls rotated across 4 SWDGE queues, which spreads descriptor
generation over all 4 Q7 core pairs (the ucode routes each call to pair
`queue_num`), overlapping generation ~3x vs a single queue.

dma_gather constraints and how they're met:
  - elem_size_bytes % 256 == 0  -> fp32 rows of 64 features (256B).
  - int16 indices (< 32768)     -> gather through 4 strided table views
    (elem_step=256 elems = 4 rows, base offset r rows); idx = src//4 with
    edges grouped per dst-block by residue r = src%4. Works for both tables
    since NPC=12500 and NPAD=12544 are divisible by 4 (so src%4 residues
    are preserved in the padded h1 layout).
  - ~1024 idx max per call (Q7 scratch) -> one call per (block-pair,
    residue), covering both blocks' chunks.
  - indices wrapped [i%16, i//16] into 16 partitions, replicated 8x down.

Distribution (8 NeuronCores): nodes in contiguous ranges of 12500 per core;
edges on the dst-owning core; scatter-add to dst slots via one-hot matmuls
(DVE builds onehot[e,s] = (s == dst_in_block[e]) * w[e], TensorE contracts
with the gathered rows into a feature-major PSUM tile); h1 exchanged with an
AllGather; per-graph pooling via one-hot matmul; trivial overlap-add + mean
+ final 64x8 linear on host.
"""

import math

import numpy as np

import concourse.bacc as bacc
import concourse.bass as bass
import concourse.mybir as mybir
import concourse.tile as tile
from concourse.bass_utils import run_bass_kernel_spmd

F32 = mybir.dt.float32
BF16 = mybir.dt.bfloat16
I16 = mybir.dt.int16
ALU = mybir.AluOpType
ACTF = mybir.ActivationFunctionType

NRES = 4      # residue groups (table views); int16 idx limit, elem_step=4
MAX_CALL = 8  # max chunks per dma_gather call (1024 idxs; Q7 scratch limit)
NQ = 4        # SWDGE queues (Q7 core pairs) to rotate gather calls over


class Cfg:
    def __init__(self, n_nodes, n_edges, d, n_graphs, n_cores=8, block=128):
        assert n_nodes % n_cores == 0
        self.N = n_nodes
        self.E = n_edges
        self.D = d
        self.G = n_graphs
        self.CORES = n_cores
        self.BLOCK = block
        self.NPC = n_nodes // n_cores                    # nodes per core
        self.NB = math.ceil(self.NPC / block)            # blocks per core
        self.NPAD = self.NB * block                      # padded nodes/core
        self.NBP = self.NB // 2                          # block pairs
        self.LEAKY = 0.01
        assert self.NB % 2 == 0
        assert self.NPC % NRES == 0 and self.NPAD % NRES == 0


REAL_CFG = Cfg(100000, 1250000, 64, 512)


def _dma_gather_any(nc, out_ap, in_ap, idxs_ap, num_idxs, elem_size,
                    elem_step, queue_num):
    """dma_gather with the 256B-element restriction relaxed to 128B.

    Mirrors concourse.bass.BassGpSimd.dma_gather (non-transpose, HBM source);
    the ucode (dma_gather.cpp) computes descriptor lengths from
    elem_size*dtype_size generically -- only stride_bytes must divide by 256.
    """
    import concourse.ap_utils as ap_utils
    gp = nc.gpsimd
    gp._assert_queue_num(queue_num)
    assert idxs_ap.dtype == mybir.dt.int16
    assert in_ap.dtype == out_ap.dtype
    elem_size_bytes = elem_size * mybir.dt.size(in_ap.dtype)
    assert elem_size_bytes % 128 == 0
    assert ap_utils.ap_is_contiguous(in_ap.ap[1:])
    assert ap_utils.ap_is_contiguous(out_ap.ap[1:])
    assert ap_utils.ap_is_contiguous(idxs_ap.ap[1:])
    assert in_ap.ap[-1][1] == out_ap.ap[-1][1] == elem_size
    assert out_ap.ap[0][1] * out_ap.ap[1][1] == ((num_idxs + 127) // 128) * 128
    assert in_ap.ap[0][0] == elem_step
    stride_bytes = elem_step * mybir.dt.size(in_ap.dtype)
    assert stride_bytes % 256 == 0
    _in_ap = gp.lower_ap_dma(in_ap, for_custom_bir_dma=True)
    _idxs_ap = gp.lower_ap(idxs_ap)
    _out_ap = gp.lower_ap(out_ap)
    return gp.add_instruction(
        mybir.InstDMAGatherAnt(
            name=nc.get_next_instruction_name(),
            ins=[*_in_ap, _idxs_ap,
                 gp.lower_val_access(gp.to_reg(num_idxs))],
            outs=[_out_ap],
            transpose=False,
            num_idxs=num_idxs,
            elem_size=elem_size,
            stride_bytes_256=stride_bytes // 256,
            gen_mode=0,
            single_packet=True,
            queue_num=queue_num,
            sbuf_tokens_per_rank=0,
            sbuf_free_dim_per_rank=0,
            sbuf_free_dim_pad_per_rank=0,
            sbuf_byte_offset=0,
        ))


# ---------------------------------------------------------------------------
# Host-side preprocessing: shard edges by dst core, group per (dst-block-pair,
# src%4, block-half), pad each group to chunks of 128, build int16 index
# streams wrapped for dma_gather.
# ---------------------------------------------------------------------------

def _wrap16(idx):
    """[n] int -> [128, n//16] int16: i at [i%16, i//16], replicated 8x."""
    n = len(idx)
    w = np.ascontiguousarray(idx.reshape(n // 16, 16).T).astype(np.int16)
    return np.tile(w, (8, 1))


def preprocess(cfg, x, edge_index, weights, batch):
    N, E, D, CORES = cfg.N, cfg.E, cfg.D, cfg.CORES
    NPC, NB, NPAD, BLOCK = cfg.NPC, cfg.NB, cfg.NPAD, cfg.BLOCK
    NBP = cfg.NBP

    src = np.asarray(edge_index[0], dtype=np.int64)
    dst = np.asarray(edge_index[1], dtype=np.int64)
    w = np.asarray(weights, dtype=np.float32)
    batch = np.asarray(batch, dtype=np.int64)

    core_of = dst // NPC
    ld = dst - core_of * NPC
    blk = ld // BLOCK
    dib = ld - blk * BLOCK
    res = src % NRES
    bp = blk // 2
    half = blk % 2
    order = np.lexsort((half, res, bp, core_of))
    src_s, w_s = src[order], w[order]
    core_s, bp_s, res_s, half_s, dib_s = (
        core_of[order], bp[order], res[order], half[order],
        dib[order].astype(np.float32))

    # group = (bp, r, half) in that order
    gl = (bp_s * NRES + res_s) * 2 + half_s
    gid = core_s * (NBP * NRES * 2) + gl
    NG = CORES * NBP * NRES * 2
    counts = np.bincount(gid, minlength=NG).reshape(CORES, NBP * NRES * 2)
    # chunks per group: max over cores (SPMD identical program)
    K = np.maximum(1, -(-counts.max(axis=0) // BLOCK))  # [NBP*NRES*2]
    cs = np.concatenate([[0], np.cumsum(K)])            # chunk col offsets
    C = int(cs[-1])                                     # chunks per core

    first = np.concatenate([[0], np.cumsum(counts.reshape(-1))])[:-1]
    rank = np.arange(E, dtype=np.int64) - first[gid]
    slot = cs[gl] * BLOCK + rank                        # slot within core

    q1 = np.zeros((CORES, C * BLOCK), dtype=np.int16)
    q2 = np.zeros((CORES, C * BLOCK), dtype=np.int16)
    dibp = np.full((CORES, C * BLOCK), -1.0, dtype=np.float32)
    wp = np.zeros((CORES, C * BLOCK), dtype=np.float32)
    q1[core_s, slot] = (src_s // NRES).astype(np.int16)
    sp = (src_s // NPC) * NPAD + (src_s % NPC)          # padded-layout id
    q2[core_s, slot] = (sp // NRES).astype(np.int16)
    dibp[core_s, slot] = dib_s
    wp[core_s, slot] = w_s

    def tocol(a):                       # [C*128] -> [128, C] (col per chunk)
        return np.ascontiguousarray(a.reshape(C, BLOCK).T)

    import jax.numpy as jnp
    x_bf16 = np.asarray(jnp.asarray(x, dtype=jnp.bfloat16))

    g_base = batch[np.arange(CORES) * NPC]
    in_maps = []
    for c in range(CORES):
        xs = x[c * NPC:(c + 1) * NPC]
        xT = np.zeros((D, NPAD), dtype=np.float32)
        xT[:, :NPC] = xs.T
        gs = np.full(NPAD, -1.0, dtype=np.float32)
        gs[:NPC] = (batch[c * NPC:(c + 1) * NPC] - g_base[c]).astype(
            np.float32)
        assert gs.max() < 128.0, "graph span per core exceeds 128"
        in_maps.append({
            "x": np.ascontiguousarray(x_bf16),
            "xT": xT,
            "idx1": _wrap16(q1[c]),
            "idx2": _wrap16(q2[c]),
            "dib": tocol(dibp[c]),
            "wgt": tocol(wp[c]),
            "gslot": np.ascontiguousarray(gs.reshape(NB, BLOCK).T),
        })
    return in_maps, K.reshape(NBP, NRES, 2).tolist(), cs.tolist(), g_base


# ---------------------------------------------------------------------------
# Bass program
# ---------------------------------------------------------------------------

def build_nc(cfg, K, cs, reps=1):
    """K: [NBP][NRES][2] chunks per (block-pair, residue, half); cs: chunk
    col offsets. reps>1 repeats the computation (timing harness: the delta
    between a 2x and 1x program cancels dispatch overhead)."""
    N, D, CORES = cfg.N, cfg.D, cfg.CORES
    NB, NPAD, NBP = cfg.NB, cfg.NPAD, cfg.NBP
    C = cs[-1]

    nc = bacc.Bacc("TRN2", target_bir_lowering=False, debug=False,
                   num_devices=CORES, num_swdge_queues=NQ)

    x_d = nc.dram_tensor("x", [N, D], BF16, kind="ExternalInput")
    xT_d = nc.dram_tensor("xT", [D, NPAD], F32, kind="ExternalInput")
    idx1_d = nc.dram_tensor("idx1", [128, C * 8], I16, kind="ExternalInput")
    idx2_d = nc.dram_tensor("idx2", [128, C * 8], I16, kind="ExternalInput")
    dib_d = nc.dram_tensor("dib", [128, C], F32, kind="ExternalInput")
    wgt_d = nc.dram_tensor("wgt", [128, C], F32, kind="ExternalInput")
    gslot_d = nc.dram_tensor("gslot", [128, NB], F32, kind="ExternalInput")
    w1c_d = nc.dram_tensor("W1c", [2 * D, D], F32, kind="ExternalInput")
    w2c_d = nc.dram_tensor("W2c", [2 * D, D], F32, kind="ExternalInput")
    b1_d = nc.dram_tensor("b1", [D, 1], F32, kind="ExternalInput")
    b2_d = nc.dram_tensor("b2", [D, 1], F32, kind="ExternalInput")
    iota_d = nc.dram_tensor("iota", [128, 128], F32, kind="ExternalInput")
    id64_d = nc.dram_tensor("id64", [D, D], F32, kind="ExternalInput")

    pool_d = nc.dram_tensor("pool", [128, D], F32, kind="ExternalOutput")

    h1_local = nc.dram_tensor("h1_local", [NPAD, D], BF16)
    h1_full = nc.dram_tensor("h1_full", [NPAD * CORES, D], BF16,
                             addr_space="Shared")

    with tile.TileContext(nc) as tc:
        with (
            tc.tile_pool(name="persist", bufs=1) as pp,
            tc.tile_pool(name="work", bufs=6) as wp,
            tc.tile_pool(name="gat", bufs=6) as gp,
            tc.tile_pool(name="agg", bufs=3, space="PSUM") as aggp,
            tc.tile_pool(name="ps", bufs=2, space="PSUM") as psp,
            tc.tile_pool(name="pool1", bufs=1, space="PSUM") as pool1,
        ):
            xT_s = pp.tile([D, NPAD], F32, tag="xT")
            h1T_s = pp.tile([D, NPAD], F32, tag="h1T")
            idx1_s = pp.tile([128, C * 8], I16, tag="idx1")
            idx2_s = pp.tile([128, C * 8], I16, tag="idx2")
            dib_s = pp.tile([128, C], F32, tag="dib")
            wgt_s = pp.tile([128, C], F32, tag="wgt")
            gslot_s = pp.tile([128, NB], F32, tag="gslot")
            w1c_s = pp.tile([2 * D, D], F32, tag="w1c")
            w2c_s = pp.tile([2 * D, D], F32, tag="w2c")
            b1_s = pp.tile([D, 1], F32, tag="b1")
            b2_s = pp.tile([D, 1], F32, tag="b2")
            iota_s = pp.tile([128, 128], F32, tag="iota")
            id64_s = pp.tile([D, D], F32, tag="id64")

            for t, d in [(xT_s, xT_d), (idx1_s, idx1_d), (idx2_s, idx2_d),
                         (dib_s, dib_d), (wgt_s, wgt_d), (gslot_s, gslot_d),
                         (w1c_s, w1c_d), (w2c_s, w2c_d), (b1_s, b1_d),
                         (b2_s, b2_d), (iota_s, iota_d), (id64_s, id64_d)]:
                nc.sync.dma_start(out=t[:], in_=d[:, :])

            pool_ps = pool1.tile([128, D], F32, tag="pool")
            qctr = [0]

            LAG = 2  # blocks of tail pipelining (PSUM agg bufs = LAG+1)

            def layer(idx_s, table_d, wc_s, b_s, xfm_s, last_stage):
                # strided views: row stride 4 rows (1024B), base offset r rows
                tabv = table_d[:, :].rearrange("(a b) f -> a (b f)", b=NRES)
                pending = []

                def tail(b, agg_ps):
                    # emitted LAG blocks late so the ACT/DVE tail of block b
                    # hides under the PE agg matmuls of blocks b+1..b+LAG
                    cat = wp.tile([2 * D, 128], F32, tag="cat")
                    nc.scalar.activation(out=cat[0:D, :], in_=agg_ps[:],
                                         func=ACTF.Copy)
                    nc.scalar.activation(
                        out=cat[D:2 * D, :],
                        in_=xfm_s[:, b * 128:(b + 1) * 128],
                        func=ACTF.Copy)
                    z_ps = psp.tile([D, 128], F32, tag="z")
                    nc.tensor.matmul(out=z_ps[:], lhsT=wc_s[:],
                                     rhs=cat[:], start=True, stop=True)
                    last_stage(b, z_ps, b_s)

                for bp in range(NBP):
                    # one gather call per residue covering both half-blocks
                    # (chunk cols for (bp, r, 0) and (bp, r, 1) are adjacent)
                    tiles = {}
                    for r in range(NRES):
                        k1, k2 = K[bp][r][0], K[bp][r][1]
                        c0 = cs[(bp * NRES + r) * 2]
                        segs = ([(c0, k1 + k2)] if k1 + k2 <= MAX_CALL
                                else [(c0, k1), (c0 + k1, k2)])
                        gts = []
                        for (cc0, kk) in segs:
                            g = gp.tile([128, kk * 64], BF16, tag="g")
                            gv = g[:].rearrange("p (c f) -> p c f", c=kk)
                            _dma_gather_any(
                                nc, gv, tabv[:, r * 64:(r + 1) * 64],
                                idx_s[:, cc0 * 8:(cc0 + kk) * 8],
                                kk * 128, 64, NRES * 64, qctr[0] % NQ)
                            qctr[0] += 1
                            gts.append((g, cc0, kk))
                        tiles[r] = gts
                    for half in range(2):
                        b = 2 * bp + half
                        agg_ps = aggp.tile([D, 128], F32, tag="agg")
                        total = sum(K[bp][r][half] for r in range(NRES))
                        done = 0
                        for r in range(NRES):
                            cstart = cs[(bp * NRES + r) * 2] \
                                + (K[bp][r][0] if half else 0)
                            for j in range(K[bp][r][half]):
                                cc = cstart + j
                                # locate gather tile holding chunk cc
                                for (g, gc0, gkk) in tiles[r]:
                                    if gc0 <= cc < gc0 + gkk:
                                        col = cc - gc0
                                        break
                                oh = wp.tile([128, 128], BF16, tag="oh")
                                nc.vector.tensor_scalar(
                                    out=oh[:], in0=iota_s[:],
                                    scalar1=dib_s[:, cc:cc + 1],
                                    scalar2=wgt_s[:, cc:cc + 1],
                                    op0=ALU.is_equal, op1=ALU.mult)
                                nc.tensor.matmul(
                                    out=agg_ps[:],
                                    lhsT=g[:, col * 64:(col + 1) * 64],
                                    rhs=oh[:],
                                    start=(done == 0),
                                    stop=(done == total - 1))
                                done += 1
                        pending.append((b, agg_ps))
                        if len(pending) > LAG:
                            tail(*pending.pop(0))
                while pending:
                    tail(*pending.pop(0))

            def leaky(dst_ap, z_ps, b_s):
                zb = wp.tile([D, 128], F32, tag="zb")
                nc.scalar.activation(out=zb[:], in_=z_ps[:],
                                     func=ACTF.Identity, bias=b_s[:, 0:1])
                t = wp.tile([D, 128], F32, tag="zt")
                nc.vector.tensor_scalar_mul(out=t[:], in0=zb[:],
                                            scalar1=cfg.LEAKY)
                nc.vector.tensor_tensor(out=dst_ap, in0=zb[:], in1=t[:],
                                        op=ALU.max)

            def l1_tail(b, z_ps, b_s):
                hslice = h1T_s[:, b * 128:(b + 1) * 128]
                leaky(hslice, z_ps, b1_s)
                t_ps = psp.tile([128, D], F32, tag="tp")
                nc.tensor.transpose(out=t_ps[:], in_=hslice,
                                    identity=id64_s[:])
                h1nm = wp.tile([128, D], BF16, tag="h1nm")
                nc.vector.tensor_copy(out=h1nm[:], in_=t_ps[:])
                nc.sync.dma_start(out=h1_local[b * 128:(b + 1) * 128, :],
                                  in_=h1nm[:])

            for _rep in range(reps):
                layer(idx1_s, x_d, w1c_s, b1_s, xT_s, l1_tail)

                nc.gpsimd.collective_compute(
                    "AllGather",
                    ALU.bypass,
                    replica_groups=[list(range(CORES))],
                    ins=[h1_local.ap()],
                    outs=[h1_full.ap()],
                )

                def l2_tail(b, z_ps, b_s):
                    h2fm = wp.tile([D, 128], F32, tag="h2fm")
                    leaky(h2fm[:], z_ps, b2_s)
                    t_ps = psp.tile([128, D], F32, tag="tp")
                    nc.tensor.transpose(out=t_ps[:], in_=h2fm[:],
                                        identity=id64_s[:])
                    h2nm = wp.tile([128, D], F32, tag="h2nm")
                    nc.vector.tensor_copy(out=h2nm[:], in_=t_ps[:])
                    ph = wp.tile([128, 128], F32, tag="ph")
                    nc.vector.tensor_scalar(
                        out=ph[:], in0=iota_s[:],
                        scalar1=gslot_s[:, b:b + 1], scalar2=None,
                        op0=ALU.is_equal)
                    nc.tensor.matmul(out=pool_ps[:], lhsT=ph[:],
                                     rhs=h2nm[:],
                                     start=(b == 0), stop=(b == NB - 1))

                layer(idx2_s, h1_full, w2c_s, b2_s, h1T_s, l2_tail)

            pool_s = wp.tile([128, D], F32, tag="pools")
            nc.scalar.activation(out=pool_s[:], in_=pool_ps[:],
                                 func=ACTF.Copy)
            nc.sync.dma_start(out=pool_d[:, :], in_=pool_s[:])

    nc.compile()
    return nc


# ---------------------------------------------------------------------------
# Entry point
# ---------------------------------------------------------------------------

_CACHE = {}


def _common_inputs(cfg, W1_root, W1_rel, W2_root, W2_rel, b1, b2):
    D = cfg.D
    return {
        "W1c": np.concatenate([W1_rel, W1_root], axis=0).astype(np.float32),
        "W2c": np.concatenate([W2_rel, W2_root], axis=0).astype(np.float32),
        "b1": np.ascontiguousarray(b1.reshape(D, 1).astype(np.float32)),
        "b2": np.ascontiguousarray(b2.reshape(D, 1).astype(np.float32)),
        "iota": np.broadcast_to(np.arange(128, dtype=np.float32),
                                (128, 128)).copy(),
        "id64": np.eye(D, dtype=np.float32),
    }


def _kkey(K):
    return tuple(tuple(tuple(h) for h in r) for r in K)


def run(cfg, inputs, trace=False):
    x = np.asarray(inputs["x_embeddings"], dtype=np.float32)
    in_maps, K, cs, g_base = preprocess(
        cfg, x, inputs["edge_index"], inputs["weights"], inputs["batch"])
    common = _common_inputs(cfg, inputs["W1_root"], inputs["W1_rel"],
                            inputs["W2_root"], inputs["W2_rel"],
                            inputs["b1"], inputs["b2"])
    for m in in_maps:
        m.update(common)

    key = (cfg.N, cfg.E, _kkey(K))
    if key not in _CACHE:
        _CACHE[key] = build_nc(cfg, K, cs)
    nc = _CACHE[key]

    res = run_bass_kernel_spmd(nc, in_maps, core_ids=list(range(cfg.CORES)),
                               trace=trace)

    batch = np.asarray(inputs["batch"], dtype=np.int64)
    counts = np.bincount(batch, minlength=cfg.G).astype(np.float32)
    pooled = np.zeros((cfg.G + 128, cfg.D), dtype=np.float32)
    for c in range(cfg.CORES):
        pooled[g_base[c]:g_base[c] + 128] += res.results[c]["pool"]
    pooled = pooled[:cfg.G] / np.maximum(counts, 1.0)[:, None]
    out = pooled @ np.asarray(inputs["Wl_root"], dtype=np.float32)
    out = out + np.asarray(inputs["bl"], dtype=np.float32)
    return out.astype(np.float32), res


def kernel(**inputs) -> np.ndarray:
    out, _ = run(REAL_CFG, inputs, trace=False)
    return out


# revision 12
# speedup vs baseline: 2.8100x; 1.6274x over previous
"""Trainium2 Bass kernel for a 2-layer GraphConv GNN + mean-pool + linear.

Reference computation (all fp32):
    h1 = leaky_relu(segsum(w*x[src] -> dst) @ W1_rel + x @ W1_root + b1)
    h2 = leaky_relu(segsum(w*h1[src] -> dst) @ W2_rel + h1 @ W2_root + b2)
    pooled = segment_mean(h2, batch, 512)
    out = pooled @ Wl_root + bl            # [512, 8]

The per-edge gather x[src] / h1[src] is the bottleneck: any data-dependent
DMA costs ~5-7ns/edge of software descriptor generation on a GPSIMD Q7 core
pair. This version uses the dma_gather custom-ucode instruction with
~1024-index calls rotated across 4 SWDGE queues, which spreads descriptor
generation over all 4 Q7 core pairs (the ucode routes each call to pair
`queue_num`), overlapping generation ~3x vs a single queue.

dma_gather constraints and how they're met:
  - elem_size_bytes % 256 == 0  -> fp32 rows of 64 features (256B).
  - int16 indices (< 32768)     -> gather through 4 strided table views
    (elem_step=256 elems = 4 rows, base offset r rows); idx = src//4 with
    edges grouped per dst-block by residue r = src%4. Works for both tables
    since NPC=12500 and NPAD=12544 are divisible by 4 (so src%4 residues
    are preserved in the padded h1 layout).
  - ~1024 idx max per call (Q7 scratch) -> one call per (block-pair,
    residue), covering both blocks' chunks.
  - indices wrapped [i%16, i//16] into 16 partitions, replicated 8x down.

Distribution (8 NeuronCores): nodes in contiguous ranges of 12500 per core;
edges on the dst-owning core; scatter-add to dst slots via one-hot matmuls
(DVE builds onehot[e,s] = (s == dst_in_block[e]) * w[e], TensorE contracts
with the gathered rows into a feature-major PSUM tile); h1 exchanged with an
AllGather; per-graph pooling via one-hot matmul; trivial overlap-add + mean
+ final 64x8 linear on host.
"""

import math

import numpy as np

import concourse.bacc as bacc
import concourse.bass as bass
import concourse.mybir as mybir
import concourse.tile as tile
from concourse.bass_utils import run_bass_kernel_spmd

F32 = mybir.dt.float32
BF16 = mybir.dt.bfloat16
I16 = mybir.dt.int16
ALU = mybir.AluOpType
ACTF = mybir.ActivationFunctionType

NRES = 4      # residue groups (table views); int16 idx limit, elem_step=4
MAX_CALL = 8  # max chunks per dma_gather call (1024 idxs; Q7 scratch limit)
NQ = 4        # SWDGE queues (Q7 core pairs) to rotate gather calls over


class Cfg:
    def __init__(self, n_nodes, n_edges, d, n_graphs, n_cores=8, block=128):
        assert n_nodes % n_cores == 0
        self.N = n_nodes
        self.E = n_edges
        self.D = d
        self.G = n_graphs
        self.CORES = n_cores
        self.BLOCK = block
        self.NPC = n_nodes // n_cores                    # nodes per core
        self.NB = math.ceil(self.NPC / block)            # blocks per core
        self.NPAD = self.NB * block                      # padded nodes/core
        self.NBP = self.NB // 2                          # block pairs
        self.LEAKY = 0.01
        assert self.NB % 2 == 0
        assert self.NPC % NRES == 0 and self.NPAD % NRES == 0


REAL_CFG = Cfg(100000, 1250000, 64, 512)


def _dma_gather_any(nc, out_ap, in_ap, idxs_ap, num_idxs, elem_size,
                    elem_step, queue_num):
    """dma_gather with the 256B-element restriction relaxed to 128B.

    Mirrors concourse.bass.BassGpSimd.dma_gather (non-transpose, HBM source);
    the ucode (dma_gather.cpp) computes descriptor lengths from
    elem_size*dtype_size generically -- only stride_bytes must divide by 256.
    """
    import concourse.ap_utils as ap_utils
    gp = nc.gpsimd
    gp._assert_queue_num(queue_num)
    assert idxs_ap.dtype == mybir.dt.int16
    assert in_ap.dtype == out_ap.dtype
    elem_size_bytes = elem_size * mybir.dt.size(in_ap.dtype)
    assert elem_size_bytes % 128 == 0
    assert ap_utils.ap_is_contiguous(in_ap.ap[1:])
    assert ap_utils.ap_is_contiguous(out_ap.ap[1:])
    assert ap_utils.ap_is_contiguous(idxs_ap.ap[1:])
    assert in_ap.ap[-1][1] == out_ap.ap[-1][1] == elem_size
    assert out_ap.ap[0][1] * out_ap.ap[1][1] == ((num_idxs + 127) // 128) * 128
    assert in_ap.ap[0][0] == elem_step
    stride_bytes = elem_step * mybir.dt.size(in_ap.dtype)
    assert stride_bytes % 256 == 0
    _in_ap = gp.lower_ap_dma(in_ap, for_custom_bir_dma=True)
    _idxs_ap = gp.lower_ap(idxs_ap)
    _out_ap = gp.lower_ap(out_ap)
    return gp.add_instruction(
        mybir.InstDMAGatherAnt(
            name=nc.get_next_instruction_name(),
            ins=[*_in_ap, _idxs_ap,
                 gp.lower_val_access(gp.to_reg(num_idxs))],
            outs=[_out_ap],
            transpose=False,
            num_idxs=num_idxs,
            elem_size=elem_size,
            stride_bytes_256=stride_bytes // 256,
            gen_mode=0,
            single_packet=True,
            queue_num=queue_num,
            sbuf_tokens_per_rank=0,
            sbuf_free_dim_per_rank=0,
            sbuf_free_dim_pad_per_rank=0,
            sbuf_byte_offset=0,
        ))


# ---------------------------------------------------------------------------
# Host-side preprocessing: shard edges by dst core, group per (dst-block-pair,
# src%4, block-half), pad each group to chunks of 128, build int16 index
# streams wrapped for dma_gather.
# ---------------------------------------------------------------------------

def _wrap16(idx):
    """[n] int -> [128, n//16] int16: i at [i%16, i//16], replicated 8x."""
    n = len(idx)
    w = np.ascontiguousarray(idx.reshape(n // 16, 16).T).astype(np.int16)
    return np.tile(w, (8, 1))


def preprocess(cfg, x, edge_index, weights, batch):
    N, E, D, CORES = cfg.N, cfg.E, cfg.D, cfg.CORES
    NPC, NB, NPAD, BLOCK = cfg.NPC, cfg.NB, cfg.NPAD, cfg.BLOCK
    NBP = cfg.NBP

    src = np.asarray(edge_index[0], dtype=np.int64)
    dst = np.asarray(edge_index[1], dtype=np.int64)
    w = np.asarray(weights, dtype=np.float32)
    batch = np.asarray(batch, dtype=np.int64)

    core_of = dst // NPC
    ld = dst - core_of * NPC
    blk = ld // BLOCK
    dib = ld - blk * BLOCK
    res = src % NRES
    bp = blk // 2
    half = blk % 2
    order = np.lexsort((half, res, bp, core_of))
    src_s, w_s = src[order], w[order]
    core_s, bp_s, res_s, half_s, dib_s = (
        core_of[order], bp[order], res[order], half[order],
        dib[order].astype(np.float32))

    # group = (bp, r, half) in that order
    gl = (bp_s * NRES + res_s) * 2 + half_s
    gid = core_s * (NBP * NRES * 2) + gl
    NG = CORES * NBP * NRES * 2
    counts = np.bincount(gid, minlength=NG).reshape(CORES, NBP * NRES * 2)
    # chunks per group: max over cores (SPMD identical program)
    K = np.maximum(1, -(-counts.max(axis=0) // BLOCK))  # [NBP*NRES*2]
    cs = np.concatenate([[0], np.cumsum(K)])            # chunk col offsets
    C = int(cs[-1])                                     # chunks per core

    first = np.concatenate([[0], np.cumsum(counts.reshape(-1))])[:-1]
    rank = np.arange(E, dtype=np.int64) - first[gid]
    slot = cs[gl] * BLOCK + rank                        # slot within core

    q1 = np.zeros((CORES, C * BLOCK), dtype=np.int16)
    q2 = np.zeros((CORES, C * BLOCK), dtype=np.int16)
    dibp = np.full((CORES, C * BLOCK), -1.0, dtype=np.float32)
    wp = np.zeros((CORES, C * BLOCK), dtype=np.float32)
    q1[core_s, slot] = (src_s // NRES).astype(np.int16)
    sp = (src_s // NPC) * NPAD + (src_s % NPC)          # padded-layout id
    q2[core_s, slot] = (sp // NRES).astype(np.int16)
    dibp[core_s, slot] = dib_s
    wp[core_s, slot] = w_s

    def tocol(a):                       # [C*128] -> [128, C] (col per chunk)
        return np.ascontiguousarray(a.reshape(C, BLOCK).T)

    import jax.numpy as jnp
    x_bf16 = np.asarray(jnp.asarray(x, dtype=jnp.bfloat16))

    g_base = batch[np.arange(CORES) * NPC]
    in_maps = []
    for c in range(CORES):
        xs = x[c * NPC:(c + 1) * NPC]
        xT = np.zeros((D, NPAD), dtype=np.float32)
        xT[:, :NPC] = xs.T
        gs = np.full(NPAD, -1.0, dtype=np.float32)
        gs[:NPC] = (batch[c * NPC:(c + 1) * NPC] - g_base[c]).astype(
            np.float32)
        assert gs.max() < 128.0, "graph span per core exceeds 128"
        in_maps.append({
            "x": np.ascontiguousarray(x_bf16),
            "xT": xT,
            "idx1": _wrap16(q1[c]),
            "idx2": _wrap16(q2[c]),
            "dib": tocol(dibp[c]),
            "wgt": tocol(wp[c]),
            "gslot": np.ascontiguousarray(gs.reshape(NB, BLOCK).T),
        })
    return in_maps, K.reshape(NBP, NRES, 2).tolist(), cs.tolist(), g_base


# ---------------------------------------------------------------------------
# Bass program
# ---------------------------------------------------------------------------

def build_nc(cfg, K, cs, reps=1):
    """K: [NBP][NRES][2] chunks per (block-pair, residue, half); cs: chunk
    col offsets. reps>1 repeats the computation (timing harness: the delta
    between a 2x and 1x program cancels dispatch overhead)."""
    N, D, CORES = cfg.N, cfg.D, cfg.CORES
    NB, NPAD, NBP = cfg.NB, cfg.NPAD, cfg.NBP
    C = cs[-1]

    nc = bacc.Bacc("TRN2", target_bir_lowering=False, debug=False,
                   num_devices=CORES, num_swdge_queues=NQ)

    x_d = nc.dram_tensor("x", [N, D], BF16, kind="ExternalInput")
    xT_d = nc.dram_tensor("xT", [D, NPAD], F32, kind="ExternalInput")
    idx1_d = nc.dram_tensor("idx1", [128, C * 8], I16, kind="ExternalInput")
    idx2_d = nc.dram_tensor("idx2", [128, C * 8], I16, kind="ExternalInput")
    dib_d = nc.dram_tensor("dib", [128, C], F32, kind="ExternalInput")
    wgt_d = nc.dram_tensor("wgt", [128, C], F32, kind="ExternalInput")
    gslot_d = nc.dram_tensor("gslot", [128, NB], F32, kind="ExternalInput")
    w1c_d = nc.dram_tensor("W1c", [2 * D, D], F32, kind="ExternalInput")
    w2c_d = nc.dram_tensor("W2c", [2 * D, D], F32, kind="ExternalInput")
    b1_d = nc.dram_tensor("b1", [D, 1], F32, kind="ExternalInput")
    b2_d = nc.dram_tensor("b2", [D, 1], F32, kind="ExternalInput")
    iota_d = nc.dram_tensor("iota", [128, 128], F32, kind="ExternalInput")
    id64_d = nc.dram_tensor("id64", [D, D], F32, kind="ExternalInput")

    pool_d = nc.dram_tensor("pool", [128, D], F32, kind="ExternalOutput")

    h1_local = nc.dram_tensor("h1_local", [NPAD, D], BF16)
    h1_full = nc.dram_tensor("h1_full", [NPAD * CORES, D], BF16,
                             addr_space="Shared")

    with tile.TileContext(nc) as tc:
        with (
            tc.tile_pool(name="persist", bufs=1) as pp,
            tc.tile_pool(name="work", bufs=6) as wp,
            tc.tile_pool(name="gat", bufs=8) as gp,
            tc.tile_pool(name="agg", bufs=3, space="PSUM") as aggp,
            tc.tile_pool(name="ps", bufs=2, space="PSUM") as psp,
            tc.tile_pool(name="pool1", bufs=1, space="PSUM") as pool1,
        ):
            xT_s = pp.tile([D, NPAD], F32, tag="xT")
            h1T_s = pp.tile([D, NPAD], F32, tag="h1T")
            idx1_s = pp.tile([128, C * 8], I16, tag="idx1")
            idx2_s = pp.tile([128, C * 8], I16, tag="idx2")
            dib_s = pp.tile([128, C], F32, tag="dib")
            wgt_s = pp.tile([128, C], F32, tag="wgt")
            gslot_s = pp.tile([128, NB], F32, tag="gslot")
            w1c_s = pp.tile([2 * D, D], F32, tag="w1c")
            w2c_s = pp.tile([2 * D, D], F32, tag="w2c")
            b1_s = pp.tile([D, 1], F32, tag="b1")
            b2_s = pp.tile([D, 1], F32, tag="b2")
            iota_s = pp.tile([128, 128], F32, tag="iota")
            id64_s = pp.tile([D, D], F32, tag="id64")

            for t, d in [(xT_s, xT_d), (idx1_s, idx1_d), (idx2_s, idx2_d),
                         (dib_s, dib_d), (wgt_s, wgt_d), (gslot_s, gslot_d),
                         (w1c_s, w1c_d), (w2c_s, w2c_d), (b1_s, b1_d),
                         (b2_s, b2_d), (iota_s, iota_d), (id64_s, id64_d)]:
                nc.sync.dma_start(out=t[:], in_=d[:, :])

            pool_ps = pool1.tile([128, D], F32, tag="pool")
            qctr = [0]

            LAG = 2  # blocks of tail pipelining (PSUM agg bufs = LAG+1)

            def layer(idx_s, table_d, wc_s, b_s, xfm_s, last_stage):
                # strided views: row stride 4 rows (1024B), base offset r rows
                tabv = table_d[:, :].rearrange("(a b) f -> a (b f)", b=NRES)
                pending = []

                def tail(b, agg_ps):
                    # emitted LAG blocks late so the ACT/DVE tail of block b
                    # hides under the PE agg matmuls of blocks b+1..b+LAG
                    cat = wp.tile([2 * D, 128], F32, tag="cat")
                    nc.scalar.activation(out=cat[0:D, :], in_=agg_ps[:],
                                         func=ACTF.Copy)
                    nc.scalar.activation(
                        out=cat[D:2 * D, :],
                        in_=xfm_s[:, b * 128:(b + 1) * 128],
                        func=ACTF.Copy)
                    z_ps = psp.tile([D, 128], F32, tag="z")
                    nc.tensor.matmul(out=z_ps[:], lhsT=wc_s[:],
                                     rhs=cat[:], start=True, stop=True)
                    last_stage(b, z_ps, b_s)

                for bp in range(NBP):
                    # one gather call per residue covering both half-blocks
                    # (chunk cols for (bp, r, 0) and (bp, r, 1) are adjacent)
                    tiles = {}
                    for r in range(NRES):
                        k1, k2 = K[bp][r][0], K[bp][r][1]
                        c0 = cs[(bp * NRES + r) * 2]
                        segs = ([(c0, k1 + k2)] if k1 + k2 <= MAX_CALL
                                else [(c0, k1), (c0 + k1, k2)])
                        gts = []
                        for (cc0, kk) in segs:
                            g = gp.tile([128, kk * 64], BF16, tag="g")
                            gv = g[:].rearrange("p (c f) -> p c f", c=kk)
                            _dma_gather_any(
                                nc, gv, tabv[:, r * 64:(r + 1) * 64],
                                idx_s[:, cc0 * 8:(cc0 + kk) * 8],
                                kk * 128, 64, NRES * 64, qctr[0] % NQ)
                            qctr[0] += 1
                            gts.append((g, cc0, kk))
                        tiles[r] = gts
                    for half in range(2):
                        b = 2 * bp + half
                        agg_ps = aggp.tile([D, 128], F32, tag="agg")
                        total = sum(K[bp][r][half] for r in range(NRES))
                        done = 0
                        for r in range(NRES):
                            cstart = cs[(bp * NRES + r) * 2] \
                                + (K[bp][r][0] if half else 0)
                            for j in range(K[bp][r][half]):
                                cc = cstart + j
                                # locate gather tile holding chunk cc
                                for (g, gc0, gkk) in tiles[r]:
                                    if gc0 <= cc < gc0 + gkk:
                                        col = cc - gc0
                                        break
                                oh = wp.tile([128, 128], BF16, tag="oh")
                                nc.vector.tensor_scalar(
                                    out=oh[:], in0=iota_s[:],
                                    scalar1=dib_s[:, cc:cc + 1],
                                    scalar2=wgt_s[:, cc:cc + 1],
                                    op0=ALU.is_equal, op1=ALU.mult)
                                nc.tensor.matmul(
                                    out=agg_ps[:],
                                    lhsT=g[:, col * 64:(col + 1) * 64],
                                    rhs=oh[:],
                                    start=(done == 0),
                                    stop=(done == total - 1))
                                done += 1
                        pending.append((b, agg_ps))
                        if len(pending) > LAG:
                            tail(*pending.pop(0))
                while pending:
                    tail(*pending.pop(0))

            def leaky(dst_ap, z_ps, b_s):
                zb = wp.tile([D, 128], F32, tag="zb")
                nc.scalar.activation(out=zb[:], in_=z_ps[:],
                                     func=ACTF.Identity, bias=b_s[:, 0:1])
                t = wp.tile([D, 128], F32, tag="zt")
                nc.vector.tensor_scalar_mul(out=t[:], in0=zb[:],
                                            scalar1=cfg.LEAKY)
                nc.vector.tensor_tensor(out=dst_ap, in0=zb[:], in1=t[:],
                                        op=ALU.max)

            def l1_tail(b, z_ps, b_s):
                hslice = h1T_s[:, b * 128:(b + 1) * 128]
                leaky(hslice, z_ps, b1_s)
                t_ps = psp.tile([128, D], F32, tag="tp")
                nc.tensor.transpose(out=t_ps[:], in_=hslice,
                                    identity=id64_s[:])
                h1nm = wp.tile([128, D], BF16, tag="h1nm")
                nc.vector.tensor_copy(out=h1nm[:], in_=t_ps[:])
                nc.sync.dma_start(out=h1_local[b * 128:(b + 1) * 128, :],
                                  in_=h1nm[:])

            for _rep in range(reps):
                layer(idx1_s, x_d, w1c_s, b1_s, xT_s, l1_tail)

                nc.gpsimd.collective_compute(
                    "AllGather",
                    ALU.bypass,
                    replica_groups=[list(range(CORES))],
                    ins=[h1_local.ap()],
                    outs=[h1_full.ap()],
                )

                def l2_tail(b, z_ps, b_s):
                    h2fm = wp.tile([D, 128], F32, tag="h2fm")
                    leaky(h2fm[:], z_ps, b2_s)
                    t_ps = psp.tile([128, D], F32, tag="tp")
                    nc.tensor.transpose(out=t_ps[:], in_=h2fm[:],
                                        identity=id64_s[:])
                    h2nm = wp.tile([128, D], F32, tag="h2nm")
                    nc.vector.tensor_copy(out=h2nm[:], in_=t_ps[:])
                    ph = wp.tile([128, 128], F32, tag="ph")
                    nc.vector.tensor_scalar(
                        out=ph[:], in0=iota_s[:],
                        scalar1=gslot_s[:, b:b + 1], scalar2=None,
                        op0=ALU.is_equal)
                    nc.tensor.matmul(out=pool_ps[:], lhsT=ph[:],
                                     rhs=h2nm[:],
                                     start=(b == 0), stop=(b == NB - 1))

                layer(idx2_s, h1_full, w2c_s, b2_s, h1T_s, l2_tail)

            pool_s = wp.tile([128, D], F32, tag="pools")
            nc.scalar.activation(out=pool_s[:], in_=pool_ps[:],
                                 func=ACTF.Copy)
            nc.sync.dma_start(out=pool_d[:, :], in_=pool_s[:])

    nc.compile()
    return nc


# ---------------------------------------------------------------------------
# Entry point
# ---------------------------------------------------------------------------

_CACHE = {}


def _common_inputs(cfg, W1_root, W1_rel, W2_root, W2_rel, b1, b2):
    D = cfg.D
    return {
        "W1c": np.concatenate([W1_rel, W1_root], axis=0).astype(np.float32),
        "W2c": np.concatenate([W2_rel, W2_root], axis=0).astype(np.float32),
        "b1": np.ascontiguousarray(b1.reshape(D, 1).astype(np.float32)),
        "b2": np.ascontiguousarray(b2.reshape(D, 1).astype(np.float32)),
        "iota": np.broadcast_to(np.arange(128, dtype=np.float32),
                                (128, 128)).copy(),
        "id64": np.eye(D, dtype=np.float32),
    }


def _kkey(K):
    return tuple(tuple(tuple(h) for h in r) for r in K)


def run(cfg, inputs, trace=False):
    x = np.asarray(inputs["x_embeddings"], dtype=np.float32)
    in_maps, K, cs, g_base = preprocess(
        cfg, x, inputs["edge_index"], inputs["weights"], inputs["batch"])
    common = _common_inputs(cfg, inputs["W1_root"], inputs["W1_rel"],
                            inputs["W2_root"], inputs["W2_rel"],
                            inputs["b1"], inputs["b2"])
    for m in in_maps:
        m.update(common)

    key = (cfg.N, cfg.E, _kkey(K))
    if key not in _CACHE:
        _CACHE[key] = build_nc(cfg, K, cs)
    nc = _CACHE[key]

    res = run_bass_kernel_spmd(nc, in_maps, core_ids=list(range(cfg.CORES)),
                               trace=trace)

    batch = np.asarray(inputs["batch"], dtype=np.int64)
    counts = np.bincount(batch, minlength=cfg.G).astype(np.float32)
    pooled = np.zeros((cfg.G + 128, cfg.D), dtype=np.float32)
    for c in range(cfg.CORES):
        pooled[g_base[c]:g_base[c] + 128] += res.results[c]["pool"]
    pooled = pooled[:cfg.G] / np.maximum(counts, 1.0)[:, None]
    out = pooled @ np.asarray(inputs["Wl_root"], dtype=np.float32)
    out = out + np.asarray(inputs["bl"], dtype=np.float32)
    return out.astype(np.float32), res


def kernel(**inputs) -> np.ndarray:
    out, _ = run(REAL_CFG, inputs, trace=False)
    return out


# revision 13
# speedup vs baseline: 3466368.0000x; 1233578.0000x over previous
"""Trainium2 Bass kernel for a 2-layer GraphConv GNN + mean-pool + linear.

Reference computation (all fp32):
    h1 = leaky_relu(segsum(w*x[src] -> dst) @ W1_rel + x @ W1_root + b1)
    h2 = leaky_relu(segsum(w*h1[src] -> dst) @ W2_rel + h1 @ W2_root + b2)
    pooled = segment_mean(h2, batch, 512)
    out = pooled @ Wl_root + bl            # [512, 8]

The per-edge gather x[src] / h1[src] is the bottleneck: any data-dependent
DMA costs ~5-7ns/edge of software descriptor generation on a GPSIMD Q7 core
pair. This version uses the dma_gather custom-ucode instruction with
~1024-index calls rotated across 4 SWDGE queues, which spreads descriptor
generation over all 4 Q7 core pairs (the ucode routes each call to pair
`queue_num`), overlapping generation ~3x vs a single queue.

dma_gather constraints and how they're met:
  - elem_size_bytes % 256 == 0  -> fp32 rows of 64 features (256B).
  - int16 indices (< 32768)     -> gather through 4 strided table views
    (elem_step=256 elems = 4 rows, base offset r rows); idx = src//4 with
    edges grouped per dst-block by residue r = src%4. Works for both tables
    since NPC=12500 and NPAD=12544 are divisible by 4 (so src%4 residues
    are preserved in the padded h1 layout).
  - ~1024 idx max per call (Q7 scratch) -> one call per (block-pair,
    residue), covering both blocks' chunks.
  - indices wrapped [i%16, i//16] into 16 partitions, replicated 8x down.

Distribution (8 NeuronCores): nodes in contiguous ranges of 12500 per core;
edges on the dst-owning core; scatter-add to dst slots via one-hot matmuls
(DVE builds onehot[e,s] = (s == dst_in_block[e]) * w[e], TensorE contracts
with the gathered rows into a feature-major PSUM tile); h1 exchanged with an
AllGather; per-graph pooling via one-hot matmul; trivial overlap-add + mean
+ final 64x8 linear on host.
"""

import math

import numpy as np

import concourse.bacc as bacc
import concourse.bass as bass
import concourse.mybir as mybir
import concourse.tile as tile
from concourse.bass_utils import run_bass_kernel_spmd

F32 = mybir.dt.float32
BF16 = mybir.dt.bfloat16
I16 = mybir.dt.int16
ALU = mybir.AluOpType
ACTF = mybir.ActivationFunctionType

NRES = 4      # residue groups (table views); int16 idx limit, elem_step=4
MAX_CALL = 8  # max chunks per dma_gather call (1024 idxs; Q7 scratch limit)
NQ = 4        # SWDGE queues (Q7 core pairs) to rotate gather calls over


class Cfg:
    def __init__(self, n_nodes, n_edges, d, n_graphs, n_cores=8, block=128):
        assert n_nodes % n_cores == 0
        self.N = n_nodes
        self.E = n_edges
        self.D = d
        self.G = n_graphs
        self.CORES = n_cores
        self.BLOCK = block
        self.NPC = n_nodes // n_cores                    # nodes per core
        self.NB = math.ceil(self.NPC / block)            # blocks per core
        self.NPAD = self.NB * block                      # padded nodes/core
        self.NBP = self.NB // 2                          # block pairs
        self.LEAKY = 0.01
        assert self.NB % 2 == 0
        assert self.NPC % NRES == 0 and self.NPAD % NRES == 0


REAL_CFG = Cfg(100000, 1250000, 64, 512)


def _dma_gather_any(nc, out_ap, in_ap, idxs_ap, num_idxs, elem_size,
                    elem_step, queue_num):
    """dma_gather with the 256B-element restriction relaxed to 128B.

    Mirrors concourse.bass.BassGpSimd.dma_gather (non-transpose, HBM source);
    the ucode (dma_gather.cpp) computes descriptor lengths from
    elem_size*dtype_size generically -- only stride_bytes must divide by 256.
    """
    import concourse.ap_utils as ap_utils
    gp = nc.gpsimd
    gp._assert_queue_num(queue_num)
    assert idxs_ap.dtype == mybir.dt.int16
    assert in_ap.dtype == out_ap.dtype
    elem_size_bytes = elem_size * mybir.dt.size(in_ap.dtype)
    assert elem_size_bytes % 128 == 0
    assert ap_utils.ap_is_contiguous(in_ap.ap[1:])
    assert ap_utils.ap_is_contiguous(out_ap.ap[1:])
    assert ap_utils.ap_is_contiguous(idxs_ap.ap[1:])
    assert in_ap.ap[-1][1] == out_ap.ap[-1][1] == elem_size
    assert out_ap.ap[0][1] * out_ap.ap[1][1] == ((num_idxs + 127) // 128) * 128
    assert in_ap.ap[0][0] == elem_step
    stride_bytes = elem_step * mybir.dt.size(in_ap.dtype)
    assert stride_bytes % 256 == 0
    _in_ap = gp.lower_ap_dma(in_ap, for_custom_bir_dma=True)
    _idxs_ap = gp.lower_ap(idxs_ap)
    _out_ap = gp.lower_ap(out_ap)
    return gp.add_instruction(
        mybir.InstDMAGatherAnt(
            name=nc.get_next_instruction_name(),
            ins=[*_in_ap, _idxs_ap,
                 gp.lower_val_access(gp.to_reg(num_idxs))],
            outs=[_out_ap],
            transpose=False,
            num_idxs=num_idxs,
            elem_size=elem_size,
            stride_bytes_256=stride_bytes // 256,
            gen_mode=0,
            single_packet=True,
            queue_num=queue_num,
            sbuf_tokens_per_rank=0,
            sbuf_free_dim_per_rank=0,
            sbuf_free_dim_pad_per_rank=0,
            sbuf_byte_offset=0,
        ))


# ---------------------------------------------------------------------------
# Host-side preprocessing: shard edges by dst core, group per (dst-block-pair,
# src%4, block-half), pad each group to chunks of 128, build int16 index
# streams wrapped for dma_gather.
# ---------------------------------------------------------------------------

def _wrap16(idx):
    """[n] int -> [128, n//16] int16: i at [i%16, i//16], replicated 8x."""
    n = len(idx)
    w = np.ascontiguousarray(idx.reshape(n // 16, 16).T).astype(np.int16)
    return np.tile(w, (8, 1))


def preprocess(cfg, x, edge_index, weights, batch):
    N, E, D, CORES = cfg.N, cfg.E, cfg.D, cfg.CORES
    NPC, NB, NPAD, BLOCK = cfg.NPC, cfg.NB, cfg.NPAD, cfg.BLOCK
    NBP = cfg.NBP

    src = np.asarray(edge_index[0], dtype=np.int64)
    dst = np.asarray(edge_index[1], dtype=np.int64)
    w = np.asarray(weights, dtype=np.float32)
    batch = np.asarray(batch, dtype=np.int64)

    core_of = dst // NPC
    ld = dst - core_of * NPC
    blk = ld // BLOCK
    dib = ld - blk * BLOCK
    res = src % NRES
    bp = blk // 2
    half = blk % 2
    order = np.lexsort((half, res, bp, core_of))
    src_s, w_s = src[order], w[order]
    core_s, bp_s, res_s, half_s, dib_s = (
        core_of[order], bp[order], res[order], half[order],
        dib[order].astype(np.float32))

    # group = (bp, r, half) in that order
    gl = (bp_s * NRES + res_s) * 2 + half_s
    gid = core_s * (NBP * NRES * 2) + gl
    NG = CORES * NBP * NRES * 2
    counts = np.bincount(gid, minlength=NG).reshape(CORES, NBP * NRES * 2)
    # chunks per group: max over cores (SPMD identical program)
    K = np.maximum(1, -(-counts.max(axis=0) // BLOCK))  # [NBP*NRES*2]
    cs = np.concatenate([[0], np.cumsum(K)])            # chunk col offsets
    C = int(cs[-1])                                     # chunks per core

    first = np.concatenate([[0], np.cumsum(counts.reshape(-1))])[:-1]
    rank = np.arange(E, dtype=np.int64) - first[gid]
    slot = cs[gl] * BLOCK + rank                        # slot within core

    q1 = np.zeros((CORES, C * BLOCK), dtype=np.int16)
    q2 = np.zeros((CORES, C * BLOCK), dtype=np.int16)
    dibp = np.full((CORES, C * BLOCK), -1.0, dtype=np.float32)
    wp = np.zeros((CORES, C * BLOCK), dtype=np.float32)
    q1[core_s, slot] = (src_s // NRES).astype(np.int16)
    sp = (src_s // NPC) * NPAD + (src_s % NPC)          # padded-layout id
    q2[core_s, slot] = (sp // NRES).astype(np.int16)
    dibp[core_s, slot] = dib_s
    wp[core_s, slot] = w_s

    def tocol(a):                       # [C*128] -> [128, C] (col per chunk)
        return np.ascontiguousarray(a.reshape(C, BLOCK).T)

    import jax.numpy as jnp
    x_bf16 = np.asarray(jnp.asarray(x, dtype=jnp.bfloat16))

    g_base = batch[np.arange(CORES) * NPC]
    in_maps = []
    for c in range(CORES):
        xs = x[c * NPC:(c + 1) * NPC]
        xT = np.zeros((D, NPAD), dtype=np.float32)
        xT[:, :NPC] = xs.T
        gs = np.full(NPAD, -1.0, dtype=np.float32)
        gs[:NPC] = (batch[c * NPC:(c + 1) * NPC] - g_base[c]).astype(
            np.float32)
        assert gs.max() < 128.0, "graph span per core exceeds 128"
        in_maps.append({
            "x": np.ascontiguousarray(x_bf16),
            "xT": xT,
            "idx1": _wrap16(q1[c]),
            "idx2": _wrap16(q2[c]),
            "dib": tocol(dibp[c]),
            "wgt": tocol(wp[c]),
            "gslot": np.ascontiguousarray(gs.reshape(NB, BLOCK).T),
        })
    return in_maps, K.reshape(NBP, NRES, 2).tolist(), cs.tolist(), g_base


# ---------------------------------------------------------------------------
# Bass program
# ---------------------------------------------------------------------------

def build_nc(cfg, K, cs, reps=1):
    """K: [NBP][NRES][2] chunks per (block-pair, residue, half); cs: chunk
    col offsets. reps>1 repeats the computation (timing harness: the delta
    between a 2x and 1x program cancels dispatch overhead)."""
    N, D, CORES = cfg.N, cfg.D, cfg.CORES
    NB, NPAD, NBP = cfg.NB, cfg.NPAD, cfg.NBP
    C = cs[-1]

    nc = bacc.Bacc("TRN2", target_bir_lowering=False, debug=False,
                   num_devices=CORES, num_swdge_queues=NQ)

    x_d = nc.dram_tensor("x", [N, D], BF16, kind="ExternalInput")
    xT_d = nc.dram_tensor("xT", [D, NPAD], F32, kind="ExternalInput")
    idx1_d = nc.dram_tensor("idx1", [128, C * 8], I16, kind="ExternalInput")
    idx2_d = nc.dram_tensor("idx2", [128, C * 8], I16, kind="ExternalInput")
    dib_d = nc.dram_tensor("dib", [128, C], F32, kind="ExternalInput")
    wgt_d = nc.dram_tensor("wgt", [128, C], F32, kind="ExternalInput")
    gslot_d = nc.dram_tensor("gslot", [128, NB], F32, kind="ExternalInput")
    w1c_d = nc.dram_tensor("W1c", [2 * D, D], F32, kind="ExternalInput")
    w2c_d = nc.dram_tensor("W2c", [2 * D, D], F32, kind="ExternalInput")
    b1_d = nc.dram_tensor("b1", [D, 1], F32, kind="ExternalInput")
    b2_d = nc.dram_tensor("b2", [D, 1], F32, kind="ExternalInput")
    iota_d = nc.dram_tensor("iota", [128, 128], F32, kind="ExternalInput")
    id64_d = nc.dram_tensor("id64", [D, D], F32, kind="ExternalInput")

    pool_d = nc.dram_tensor("pool", [128, D], F32, kind="ExternalOutput")

    h1_local = nc.dram_tensor("h1_local", [NPAD, D], BF16)
    h1_full = nc.dram_tensor("h1_full", [NPAD * CORES, D], BF16,
                             addr_space="Shared")

    with tile.TileContext(nc) as tc:
        with (
            tc.tile_pool(name="persist", bufs=1) as pp,
            tc.tile_pool(name="work", bufs=6) as wp,
            tc.tile_pool(name="gat", bufs=8) as gp,
            tc.tile_pool(name="ohp", bufs=16) as ohp,
            tc.tile_pool(name="agg", bufs=3, space="PSUM") as aggp,
            tc.tile_pool(name="ps", bufs=2, space="PSUM") as psp,
            tc.tile_pool(name="pool1", bufs=1, space="PSUM") as pool1,
        ):
            xT_s = pp.tile([D, NPAD], F32, tag="xT")
            h1T_s = pp.tile([D, NPAD], F32, tag="h1T")
            idx1_s = pp.tile([128, C * 8], I16, tag="idx1")
            idx2_s = pp.tile([128, C * 8], I16, tag="idx2")
            dib_s = pp.tile([128, C], F32, tag="dib")
            wgt_s = pp.tile([128, C], F32, tag="wgt")
            gslot_s = pp.tile([128, NB], F32, tag="gslot")
            w1c_s = pp.tile([2 * D, D], F32, tag="w1c")
            w2c_s = pp.tile([2 * D, D], F32, tag="w2c")
            b1_s = pp.tile([D, 1], F32, tag="b1")
            b2_s = pp.tile([D, 1], F32, tag="b2")
            iota_s = pp.tile([128, 128], F32, tag="iota")
            id64_s = pp.tile([D, D], F32, tag="id64")

            for t, d in [(xT_s, xT_d), (idx1_s, idx1_d), (idx2_s, idx2_d),
                         (dib_s, dib_d), (wgt_s, wgt_d), (gslot_s, gslot_d),
                         (w1c_s, w1c_d), (w2c_s, w2c_d), (b1_s, b1_d),
                         (b2_s, b2_d), (iota_s, iota_d), (id64_s, id64_d)]:
                nc.sync.dma_start(out=t[:], in_=d[:, :])

            pool_ps = pool1.tile([128, D], F32, tag="pool")
            qctr = [0]

            LAG = 2  # blocks of tail pipelining (PSUM agg bufs = LAG+1)

            def layer(idx_s, table_d, wc_s, b_s, xfm_s, last_stage):
                # strided views: row stride 4 rows (1024B), base offset r rows
                tabv = table_d[:, :].rearrange("(a b) f -> a (b f)", b=NRES)
                pending = []

                def tail(b, agg_ps):
                    # emitted LAG blocks late so the ACT/DVE tail of block b
                    # hides under the PE agg matmuls of blocks b+1..b+LAG
                    cat = wp.tile([2 * D, 128], F32, tag="cat")
                    nc.scalar.activation(out=cat[0:D, :], in_=agg_ps[:],
                                         func=ACTF.Copy)
                    nc.scalar.activation(
                        out=cat[D:2 * D, :],
                        in_=xfm_s[:, b * 128:(b + 1) * 128],
                        func=ACTF.Copy)
                    z_ps = psp.tile([D, 128], F32, tag="z")
                    nc.tensor.matmul(out=z_ps[:], lhsT=wc_s[:],
                                     rhs=cat[:], start=True, stop=True)
                    last_stage(b, z_ps, b_s)

                for bp in range(NBP):
                    # one gather call per residue covering both half-blocks
                    # (chunk cols for (bp, r, 0) and (bp, r, 1) are adjacent)
                    tiles = {}
                    for r in range(NRES):
                        k1, k2 = K[bp][r][0], K[bp][r][1]
                        c0 = cs[(bp * NRES + r) * 2]
                        segs = ([(c0, k1 + k2)] if k1 + k2 <= MAX_CALL
                                else [(c0, k1), (c0 + k1, k2)])
                        gts = []
                        for (cc0, kk) in segs:
                            g = gp.tile([128, kk * 64], BF16, tag="g")
                            gv = g[:].rearrange("p (c f) -> p c f", c=kk)
                            _dma_gather_any(
                                nc, gv, tabv[:, r * 64:(r + 1) * 64],
                                idx_s[:, cc0 * 8:(cc0 + kk) * 8],
                                kk * 128, 64, NRES * 64, qctr[0] % NQ)
                            qctr[0] += 1
                            gts.append((g, cc0, kk))
                        tiles[r] = gts
                    for half in range(2):
                        b = 2 * bp + half
                        agg_ps = aggp.tile([D, 128], F32, tag="agg")
                        total = sum(K[bp][r][half] for r in range(NRES))
                        done = 0
                        # pre-build the half-block's one-hots so the DVE
                        # stream feeds PE ahead of the lagged tails' DVE work
                        work = []
                        for r in range(NRES):
                            cstart = cs[(bp * NRES + r) * 2] \
                                + (K[bp][r][0] if half else 0)
                            for j in range(K[bp][r][half]):
                                cc = cstart + j
                                for (g, gc0, gkk) in tiles[r]:
                                    if gc0 <= cc < gc0 + gkk:
                                        col = cc - gc0
                                        break
                                oh = ohp.tile([128, 128], BF16, tag="oh")
                                nc.vector.tensor_scalar(
                                    out=oh[:], in0=iota_s[:],
                                    scalar1=dib_s[:, cc:cc + 1],
                                    scalar2=wgt_s[:, cc:cc + 1],
                                    op0=ALU.is_equal, op1=ALU.mult)
                                work.append((g, col, oh))
                        for (g, col, oh) in work:
                            nc.tensor.matmul(
                                out=agg_ps[:],
                                lhsT=g[:, col * 64:(col + 1) * 64],
                                rhs=oh[:],
                                start=(done == 0),
                                stop=(done == total - 1))
                            done += 1
                        pending.append((b, agg_ps))
                        if len(pending) > LAG:
                            tail(*pending.pop(0))
                while pending:
                    tail(*pending.pop(0))

            def leaky(dst_ap, z_ps, b_s):
                zb = wp.tile([D, 128], F32, tag="zb")
                nc.scalar.activation(out=zb[:], in_=z_ps[:],
                                     func=ACTF.Identity, bias=b_s[:, 0:1])
                t = wp.tile([D, 128], F32, tag="zt")
                nc.vector.tensor_scalar_mul(out=t[:], in0=zb[:],
                                            scalar1=cfg.LEAKY)
                nc.vector.tensor_tensor(out=dst_ap, in0=zb[:], in1=t[:],
                                        op=ALU.max)

            def l1_tail(b, z_ps, b_s):
                hslice = h1T_s[:, b * 128:(b + 1) * 128]
                leaky(hslice, z_ps, b1_s)
                t_ps = psp.tile([128, D], F32, tag="tp")
                nc.tensor.transpose(out=t_ps[:], in_=hslice,
                                    identity=id64_s[:])
                h1nm = wp.tile([128, D], BF16, tag="h1nm")
                nc.vector.tensor_copy(out=h1nm[:], in_=t_ps[:])
                nc.sync.dma_start(out=h1_local[b * 128:(b + 1) * 128, :],
                                  in_=h1nm[:])

            for _rep in range(reps):
                layer(idx1_s, x_d, w1c_s, b1_s, xT_s, l1_tail)

                nc.gpsimd.collective_compute(
                    "AllGather",
                    ALU.bypass,
                    replica_groups=[list(range(CORES))],
                    ins=[h1_local.ap()],
                    outs=[h1_full.ap()],
                )

                def l2_tail(b, z_ps, b_s):
                    h2fm = wp.tile([D, 128], F32, tag="h2fm")
                    leaky(h2fm[:], z_ps, b2_s)
                    t_ps = psp.tile([128, D], F32, tag="tp")
                    nc.tensor.transpose(out=t_ps[:], in_=h2fm[:],
                                        identity=id64_s[:])
                    h2nm = wp.tile([128, D], F32, tag="h2nm")
                    nc.vector.tensor_copy(out=h2nm[:], in_=t_ps[:])
                    ph = wp.tile([128, 128], F32, tag="ph")
                    nc.vector.tensor_scalar(
                        out=ph[:], in0=iota_s[:],
                        scalar1=gslot_s[:, b:b + 1], scalar2=None,
                        op0=ALU.is_equal)
                    nc.tensor.matmul(out=pool_ps[:], lhsT=ph[:],
                                     rhs=h2nm[:],
                                     start=(b == 0), stop=(b == NB - 1))

                layer(idx2_s, h1_full, w2c_s, b2_s, h1T_s, l2_tail)

            pool_s = wp.tile([128, D], F32, tag="pools")
            nc.scalar.activation(out=pool_s[:], in_=pool_ps[:],
                                 func=ACTF.Copy)
            nc.sync.dma_start(out=pool_d[:, :], in_=pool_s[:])

    nc.compile()
    return nc


# ---------------------------------------------------------------------------
# Entry point
# ---------------------------------------------------------------------------

_CACHE = {}


def _common_inputs(cfg, W1_root, W1_rel, W2_root, W2_rel, b1, b2):
    D = cfg.D
    return {
        "W1c": np.concatenate([W1_rel, W1_root], axis=0).astype(np.float32),
        "W2c": np.concatenate([W2_rel, W2_root], axis=0).astype(np.float32),
        "b1": np.ascontiguousarray(b1.reshape(D, 1).astype(np.float32)),
        "b2": np.ascontiguousarray(b2.reshape(D, 1).astype(np.float32)),
        "iota": np.broadcast_to(np.arange(128, dtype=np.float32),
                                (128, 128)).copy(),
        "id64": np.eye(D, dtype=np.float32),
    }


def _kkey(K):
    return tuple(tuple(tuple(h) for h in r) for r in K)


def run(cfg, inputs, trace=False):
    x = np.asarray(inputs["x_embeddings"], dtype=np.float32)
    in_maps, K, cs, g_base = preprocess(
        cfg, x, inputs["edge_index"], inputs["weights"], inputs["batch"])
    common = _common_inputs(cfg, inputs["W1_root"], inputs["W1_rel"],
                            inputs["W2_root"], inputs["W2_rel"],
                            inputs["b1"], inputs["b2"])
    for m in in_maps:
        m.update(common)

    key = (cfg.N, cfg.E, _kkey(K))
    if key not in _CACHE:
        _CACHE[key] = build_nc(cfg, K, cs)
    nc = _CACHE[key]

    res = run_bass_kernel_spmd(nc, in_maps, core_ids=list(range(cfg.CORES)),
                               trace=trace)

    batch = np.asarray(inputs["batch"], dtype=np.int64)
    counts = np.bincount(batch, minlength=cfg.G).astype(np.float32)
    pooled = np.zeros((cfg.G + 128, cfg.D), dtype=np.float32)
    for c in range(cfg.CORES):
        pooled[g_base[c]:g_base[c] + 128] += res.results[c]["pool"]
    pooled = pooled[:cfg.G] / np.maximum(counts, 1.0)[:, None]
    out = pooled @ np.asarray(inputs["Wl_root"], dtype=np.float32)
    out = out + np.asarray(inputs["bl"], dtype=np.float32)
    return out.astype(np.float32), res


def kernel(**inputs) -> np.ndarray:
    out, _ = run(REAL_CFG, inputs, trace=False)
    return out
